# revision 13
# baseline (speedup 1.0000x reference)
"""Trainium2 Bass kernel for a GroupNorm + 8-head spatial self-attention block.

Strategy (8 cores): shard the 16 (batch, head) pairs -> each core handles one
batch b = core//4 and a pair of heads ho = (core%4)*2. Each core:
  GN(x_b) -> q/k/v for its 2 heads (128 channels) -> attention (S^T layout,
  exp on ScalarE from PSUM, O^T via [v^T | ones] matmuls giving the softmax
  denominator for free) -> partial output projection wo[:, its 128 cols] @ O.
Host sums the 4 partials per batch, adds bo and the residual.
"""

import sys

if "/opt/trn_rl_repo" not in sys.path:
    sys.path.insert(0, "/opt/trn_rl_repo")

import numpy as np

B, C, H, W = 2, 512, 64, 64
N = H * W              # 4096 tokens
NH, HD = 8, 64         # heads, head_dim
NG = 32                # groupnorm groups (16 channels each)
EPS = 1e-5
N_CORES = 8
P = 128                # partitions
IT = 512               # i-tile (query) width
N_IT = N // IT         # 8
N_JB = N // P          # 32 j-blocks
GROUPS = [3] * 10 + [2]  # jb group sizes per exp batch (sum = 32)


def build_module(do_compile=True, mode=None):
    import os
    import concourse.bass as bass
    import concourse.mybir as mybir
    import concourse.tile as tile
    from concourse import bacc
    from concourse.masks import make_identity

    if mode is None:
        mode = os.environ.get("XATTN_MODE", "bf16")
    assert mode in ("fp32", "bf16")

    f32 = mybir.dt.float32
    # compute dtype for matmul operands: bf16 runs the PE at 1 cycle/row
    # (fp32 matmuls cost 4 cycles/row); accumulation stays fp32 in PSUM.
    dt_c = mybir.dt.bfloat16 if mode == "bf16" else f32
    AF = mybir.ActivationFunctionType
    ALU = mybir.AluOpType

    def mm(out, lhsT, rhs, **kw):
        nc.tensor.matmul(out, lhsT, rhs, **kw)

    nc = bacc.Bacc(name=f"xattn_{mode}")

    x_d = nc.dram_tensor("x", (C, N), f32, kind="ExternalInput")
    wqT_d = nc.dram_tensor("wqT", (C, P), dt_c, kind="ExternalInput")
    wkvT_d = nc.dram_tensor("wkvT", (C, 2 * P), dt_c, kind="ExternalInput")
    woTA_d = nc.dram_tensor("woTA", (HD, C), dt_c, kind="ExternalInput")
    woTB_d = nc.dram_tensor("woTB", (HD, C), dt_c, kind="ExternalInput")
    bq_d = nc.dram_tensor("bq", (P,), f32, kind="ExternalInput")
    bk_d = nc.dram_tensor("bk", (P,), f32, kind="ExternalInput")
    bv_d = nc.dram_tensor("bv", (P,), f32, kind="ExternalInput")
    gnw_d = nc.dram_tensor("gnw", (C,), f32, kind="ExternalInput")
    gnb_d = nc.dram_tensor("gnb", (C,), f32, kind="ExternalInput")
    gmask_d = nc.dram_tensor("gmask", (4, P, NG), f32, kind="ExternalInput")
    gmaskT_d = nc.dram_tensor("gmaskT", (4, NG, P), f32, kind="ExternalInput")
    out_d = nc.dram_tensor("out", (C, N), f32, kind="ExternalOutput")

    with tile.TileContext(nc) as tc:
        with (
            tc.tile_pool(name="const", bufs=1) as const,
            tc.tile_pool(name="qkv", bufs=1) as qkvp,
            tc.tile_pool(name="vt", bufs=1) as vtp,
            tc.tile_pool(name="stx", bufs=1, space="PSUM") as stx,
            tc.tile_pool(name="sty", bufs=1, space="PSUM") as sty,
            tc.tile_pool(name="oa", bufs=1, space="PSUM") as oap,
            tc.tile_pool(name="ob", bufs=1, space="PSUM") as obp,
            tc.tile_pool(name="pta", bufs=3) as pta,
            tc.tile_pool(name="ptb", bufs=3) as ptb,
            tc.tile_pool(name="itn", bufs=2) as itn,
            tc.tile_pool(name="dram", bufs=2, space="DRAM") as dramp,
        ):
            # ---------- constants ----------
            wqT_sb = const.tile([P, 4, P], dt_c, tag="wq")
            nc.sync.dma_start(wqT_sb[:], wqT_d[:].rearrange("(t p) m -> p t m", p=P))
            wkvT_sb = const.tile([P, 4, 2 * P], dt_c, tag="wkv")
            nc.sync.dma_start(wkvT_sb[:], wkvT_d[:].rearrange("(t p) m -> p t m", p=P))
            woTA_sb = const.tile([HD, C], dt_c, tag="woa")
            nc.sync.dma_start(woTA_sb[:], woTA_d[:])
            woTB_sb = const.tile([HD, C], dt_c, tag="wob")
            nc.sync.dma_start(woTB_sb[:], woTB_d[:])
            bq_sb = const.tile([P, 1], f32, tag="bq")
            nc.sync.dma_start(bq_sb[:], bq_d[:, None])
            bk_sb = const.tile([P, 1], f32, tag="bk")
            nc.sync.dma_start(bk_sb[:], bk_d[:, None])
            bv_sb = const.tile([P, 1], f32, tag="bv")
            nc.sync.dma_start(bv_sb[:], bv_d[:, None])
            gnw_sb = const.tile([P, 4], f32, tag="gnw")
            nc.sync.dma_start(gnw_sb[:], gnw_d[:].rearrange("(t p) -> p t", p=P))
            gnb_sb = const.tile([P, 4], f32, tag="gnb")
            nc.sync.dma_start(gnb_sb[:], gnb_d[:].rearrange("(t p) -> p t", p=P))
            gmask_sb = const.tile([P, 4, NG], f32, tag="gmask")
            nc.sync.dma_start(gmask_sb[:], gmask_d[:].rearrange("t p g -> p t g"))
            gmaskT_sb = const.tile([NG, 4, P], f32, tag="gmaskT")
            nc.sync.dma_start(gmaskT_sb[:], gmaskT_d[:].rearrange("t k m -> k t m"))
            ident_sb = const.tile([P, P], dt_c, tag="ident")
            make_identity(nc, ident_sb[:])
            eps_sb = const.tile([NG, 1], f32, tag="eps")
            nc.vector.memset(eps_sb[:], EPS)
            warm = const.tile([1, 2], f32, tag="warm")
            nc.vector.memset(warm[:], 1.0)
            nc.scalar.activation(warm[:, 0:1], warm[:, 0:1], AF.Exp)
            nc.scalar.activation(warm[:, 1:2], warm[:, 1:2], AF.Ln)

            q_sb = qkvp.tile([P, N], dt_c, tag="q")
            k_sb = qkvp.tile([P, N], dt_c, tag="k")
            v_sb = qkvp.tile([P, N], dt_c, tag="v")

            # ---------- phase 1: GroupNorm + QKV projections ----------
            with tc.tile_pool(name="xp", bufs=1) as xp, \
                 tc.tile_pool(name="gn", bufs=1) as gnp:
                x_t = [xp.tile([P, N], f32, tag=f"x{t}", name=f"x{t}")
                       for t in range(4)]
                if mode == "bf16":
                    xs_t = [xp.tile([P, N], dt_c, tag=f"xs{t}", name=f"xs{t}")
                            for t in range(4)]
                else:
                    xs_t = x_t  # normalize in place, fp32 matmul operands
                for t in range(4):
                    nc.sync.dma_start(x_t[t][:], x_d[t * P:(t + 1) * P, :])

                # per-channel stats via bn_stats/bn_aggr (8 subgroups of 512)
                FM = 512
                nsub = N // FM
                BSD = nc.vector.BN_STATS_DIM
                grp_ps = oap.tile([NG, 2], f32, tag="oa")
                for t in range(4):
                    stats = gnp.tile([P, nsub, BSD], f32, tag="bnst")
                    xr = x_t[t][:].rearrange("p (s f) -> p s f", f=FM)
                    for s in range(nsub):
                        nc.vector.bn_stats(stats[:, s, :], xr[:, s, :])
                    mv = gnp.tile([P, 2], f32, tag="mv")
                    nc.vector.bn_aggr(mv[:], stats[:])
                    # stats2: col0 = mean, col1 = E[x^2] = var + mean^2
                    st2 = gnp.tile([P, 2], f32, tag="st2")
                    nc.vector.tensor_mul(st2[:, 1:2], mv[:, 0:1], mv[:, 0:1])
                    nc.vector.tensor_add(st2[:, 1:2], st2[:, 1:2], mv[:, 1:2])
                    nc.vector.tensor_copy(st2[:, 0:1], mv[:, 0:1])
                    # group aggregate: [NG, 2] += gmask_t.T @ st2  (mask = 1/16)
                    nc.tensor.matmul(grp_ps[:], gmask_sb[:, t, :], st2[:],
                                     start=(t == 0), stop=(t == 3))

                # group mu / rstd
                gmv = gnp.tile([NG, 2], f32, tag="gmv")
                nc.vector.tensor_copy(gmv[:], grp_ps[:])
                varg = gnp.tile([NG, 1], f32, tag="varg")
                nc.vector.tensor_mul(varg[:], gmv[:, 0:1], gmv[:, 0:1])
                nc.vector.tensor_tensor(varg[:], gmv[:, 1:2], varg[:],
                                        op=ALU.subtract)
                # rstd = exp(-0.5 * ln(var + eps))
                gvals = gnp.tile([NG, 2], f32, tag="gvals")
                nc.scalar.activation(varg[:], varg[:], AF.Ln, bias=eps_sb[:])
                nc.scalar.activation(gvals[:, 1:2], varg[:], AF.Exp, scale=-0.5)
                nc.vector.tensor_copy(gvals[:, 0:1], gmv[:, 0:1])

                # scatter back to channels: chan = gmaskT_t.T @ gvals -> [128, 2]
                for t in range(4):
                    chan_ps = obp.tile([P, 2], f32, tag="ob")
                    nc.tensor.matmul(chan_ps[:], gmaskT_sb[:, t, :], gvals[:],
                                     start=True, stop=True)
                    s_t = gnp.tile([P, 1], f32, tag=f"s{t}")
                    t_t = gnp.tile([P, 1], f32, tag=f"t{t}")
                    nc.vector.tensor_mul(s_t[:], chan_ps[:, 1:2], gnw_sb[:, t:t + 1])
                    nc.vector.tensor_mul(t_t[:], chan_ps[:, 0:1], s_t[:])
                    nc.vector.tensor_tensor(t_t[:], gnb_sb[:, t:t + 1], t_t[:],
                                            op=ALU.subtract)
                    # xs = x * s + t   (the full normalized+affine input),
                    # cast to the matmul compute dtype on write
                    nc.vector.tensor_scalar(xs_t[t][:], x_t[t][:], s_t[:], t_t[:],
                                            op0=ALU.mult, op1=ALU.add)

                # QKV projections: q = wqT.T @ xs + bq ; [k|v] = wkvT.T @ xs + b
                for n in range(N_IT):
                    nsl = slice(n * IT, (n + 1) * IT)
                    q_ps = (stx if n % 2 == 0 else sty).tile(
                        [P, 3, IT], f32, tag="stx" if n % 2 == 0 else "sty")
                    for t in range(4):
                        mm(q_ps[:, 0, :], wqT_sb[:, t, :],
                           xs_t[t][:, nsl], start=(t == 0), stop=(t == 3))
                    nc.vector.tensor_scalar_add(q_sb[:, nsl], q_ps[:, 0, :],
                                                bq_sb[:])
                for n in range(N_IT):
                    nsl = slice(n * IT, (n + 1) * IT)
                    kv_ps = (stx if n % 2 == 0 else sty).tile(
                        [P, 3, IT], f32, tag="stx" if n % 2 == 0 else "sty")
                    for t in range(4):
                        mm(kv_ps[:, 0, :], wkvT_sb[:, t, 0:P],
                           xs_t[t][:, nsl], start=(t == 0), stop=(t == 3))
                        mm(kv_ps[:, 1, :], wkvT_sb[:, t, P:2 * P],
                           xs_t[t][:, nsl], start=(t == 0), stop=(t == 3))
                    nc.vector.tensor_scalar_add(k_sb[:, nsl], kv_ps[:, 0, :],
                                                bk_sb[:])
                    nc.vector.tensor_scalar_add(v_sb[:, nsl], kv_ps[:, 1, :],
                                                bv_sb[:])

            # ---------- v^T (with trailing ones column for the denominator) ----
            vTA_sb = vtp.tile([P, N_JB, HD + 1], dt_c, tag="vta")
            vTB_sb = vtp.tile([P, N_JB, HD + 1], dt_c, tag="vtb")
            nc.vector.memset(vTA_sb[:, :, HD:HD + 1], 1.0)
            nc.vector.memset(vTB_sb[:, :, HD:HD + 1], 1.0)
            for jb in range(N_JB):
                tp_ps = (oap if jb % 2 == 0 else obp).tile(
                    [P, P], dt_c, tag="oa" if jb % 2 == 0 else "ob")
                nc.tensor.transpose(tp_ps[:], v_sb[:, jb * P:(jb + 1) * P],
                                    ident_sb[:])
                nc.vector.tensor_copy(vTA_sb[:, jb, 0:HD], tp_ps[:, 0:HD])
                nc.vector.tensor_copy(vTB_sb[:, jb, 0:HD], tp_ps[:, HD:P])

            # ---------- phase 2: attention ----------
            for it in range(N_IT):
                isl = slice(it * IT, (it + 1) * IT)
                oa_t = oap.tile([P, IT], f32, tag="oa")
                ob_t = obp.tile([P, IT], f32, tag="ob")
                jb0 = 0
                for gs in GROUPS:
                    st_a = stx.tile([P, 3, IT], f32, tag="stx")
                    st_b = sty.tile([P, 3, IT], f32, tag="sty")
                    for jj in range(gs):
                        jb = jb0 + jj
                        jsl = slice(jb * P, (jb + 1) * P)
                        mm(st_a[:, jj, :], k_sb[0:HD, jsl],
                           q_sb[0:HD, isl], start=True, stop=True)
                        mm(st_b[:, jj, :], k_sb[HD:P, jsl],
                           q_sb[HD:P, isl], start=True, stop=True,
                           tile_position=(64, 0))
                    pa = pta.tile([P, 3, IT], dt_c, tag="pta")
                    pb = ptb.tile([P, 3, IT], dt_c, tag="ptb")
                    nc.scalar.activation(pa[:, 0:gs, :], st_a[:, 0:gs, :],
                                         AF.Exp, scale=0.125)
                    nc.scalar.activation(pb[:, 0:gs, :], st_b[:, 0:gs, :],
                                         AF.Exp, scale=0.125)
                    for jj in range(gs):
                        jb = jb0 + jj
                        mm(oa_t[0:HD + 1, :], vTA_sb[:, jb, :],
                           pa[:, jj, :], start=(jb == 0),
                           stop=(jb == N_JB - 1))
                        mm(ob_t[0:HD + 1, :], vTB_sb[:, jb, :],
                           pb[:, jj, :], start=(jb == 0),
                           stop=(jb == N_JB - 1))
                    jb0 += gs

                # normalization: denominators sit in row HD of oa_t / ob_t
                denA = itn.tile([HD + 1, IT], f32, tag="denA")
                denB = itn.tile([HD + 1, IT], f32, tag="denB")
                ostA = itn.tile([HD, IT], dt_c, tag="ostA")
                ostB = itn.tile([HD, IT], dt_c, tag="ostB")
                nc.vector.tensor_copy(denA[HD:HD + 1, :], oa_t[HD:HD + 1, :])
                nc.vector.tensor_copy(denB[HD:HD + 1, :], ob_t[HD:HD + 1, :])
                nc.vector.reciprocal(denA[HD:HD + 1, :], denA[HD:HD + 1, :])
                nc.vector.reciprocal(denB[HD:HD + 1, :], denB[HD:HD + 1, :])
                scr = dramp.tile([2, IT], f32, tag="scr")
                nc.sync.dma_start(scr[0:1, :], denA[HD:HD + 1, :])
                nc.sync.dma_start(scr[1:2, :], denB[HD:HD + 1, :])
                import concourse.bass as _b
                bcA = itn.tile([HD, IT], f32, tag="bcA")
                bcB = itn.tile([HD, IT], f32, tag="bcB")
                srcA = _b.AP(tensor=scr.tensor, offset=scr.offset,
                             ap=[[0, HD], [1, IT]])
                srcB = _b.AP(tensor=scr.tensor, offset=scr.offset + IT,
                             ap=[[0, HD], [1, IT]])
                nc.sync.dma_start(bcA[:], srcA)
                nc.sync.dma_start(bcB[:], srcB)
                nc.vector.tensor_mul(ostA[:], oa_t[0:HD, :], bcA[:])
                nc.vector.tensor_mul(ostB[:], ob_t[0:HD, :], bcB[:])

                # output projection (K=64 per head, accumulate both heads)
                for mt in range(4):
                    msl = slice(mt * P, (mt + 1) * P)
                    pr_ps = (oap if mt % 2 == 0 else obp).tile(
                        [P, IT], f32, tag="oa" if mt % 2 == 0 else "ob")
                    mm(pr_ps[:], woTA_sb[:, msl], ostA[:],
                       start=True, stop=False)
                    mm(pr_ps[:], woTB_sb[:, msl], ostB[:],
                       start=False, stop=True)
                    pr_sb = itn.tile([P, IT], f32, tag="prsb")
                    nc.vector.tensor_copy(pr_sb[:], pr_ps[:])
                    nc.sync.dma_start(out_d[msl, isl], pr_sb[:])

    if do_compile:
        nc.compile()
    return nc


_CACHE = {}


def _get_runner():
    if "runner" in _CACHE:
        return _CACHE["runner"]
    from concourse.bass_utils import run_bass_kernel_spmd

    nc = build_module()

    def run(in_maps):
        res = run_bass_kernel_spmd(nc, in_maps, core_ids=list(range(N_CORES)))
        return [r["out"] for r in res.results]

    _CACHE["runner"] = run
    return run


def _masks():
    gmask = np.zeros((4, P, NG), np.float32)
    gmaskT = np.zeros((4, NG, P), np.float32)
    for t in range(4):
        for p in range(P):
            g = (t * P + p) // 16
            gmask[t, p, g] = 1.0 / 16.0
            gmaskT[t, g, p] = 1.0
    return gmask, gmaskT


def kernel(x, gn_w, gn_b, wq, bq, wkv, bkv, wo, bo):
    x = np.asarray(x, np.float32)
    gn_w = np.asarray(gn_w, np.float32)
    gn_b = np.asarray(gn_b, np.float32)
    wq = np.asarray(wq, np.float32)
    bq = np.asarray(bq, np.float32)
    wkv = np.asarray(wkv, np.float32)
    bkv = np.asarray(bkv, np.float32)
    wo = np.asarray(wo, np.float32)
    bo = np.asarray(bo, np.float32)

    import os
    import ml_dtypes
    wdt = (np.dtype(ml_dtypes.bfloat16)
           if os.environ.get("XATTN_MODE", "bf16") == "bf16" else np.float32)
    gmask, gmaskT = _masks()
    xf = x.reshape(B, C, N)
    in_maps = []
    for core in range(N_CORES):
        b = core // 4
        ho = (core % 4) * 2
        rows = slice(ho * HD, ho * HD + P)
        wkv_h = np.concatenate([wkv[ho * HD:ho * HD + P, :],
                                wkv[C + ho * HD:C + ho * HD + P, :]], axis=0)
        wo_h = wo[:, rows]  # (C, 128)
        in_maps.append({
            "x": np.ascontiguousarray(xf[b]),
            "wqT": np.ascontiguousarray(wq[rows, :].T).astype(wdt),
            "wkvT": np.ascontiguousarray(wkv_h.T).astype(wdt),
            "woTA": np.ascontiguousarray(wo_h[:, 0:HD].T).astype(wdt),
            "woTB": np.ascontiguousarray(wo_h[:, HD:P].T).astype(wdt),
            "bq": np.ascontiguousarray(bq[rows]),
            "bk": np.ascontiguousarray(bkv[ho * HD:ho * HD + P]),
            "bv": np.ascontiguousarray(bkv[C + ho * HD:C + ho * HD + P]),
            "gnw": gn_w, "gnb": gn_b,
            "gmask": gmask, "gmaskT": gmaskT,
        })

    partials = _get_runner()(in_maps)
    out = np.empty((B, C, N), np.float32)
    for b in range(B):
        acc = partials[4 * b].astype(np.float32)
        for c in range(1, 4):
            acc = acc + partials[4 * b + c]
        out[b] = acc + bo[:, None] + xf[b]
    return out.reshape(B, C, H, W)


# revision 15
# speedup vs baseline: 2.9223x; 2.9223x over previous
"""Trainium2 Bass kernel for a GroupNorm + 8-head spatial self-attention block.

Strategy (8 cores): shard the 16 (batch, head) pairs -> each core handles one
batch b = core//4 and a pair of heads ho = (core%4)*2. Each core:
  GN(x_b) -> q/k/v for its 2 heads (128 channels) -> attention (S^T layout,
  exp on ScalarE from PSUM, O^T via [v^T | ones] matmuls giving the softmax
  denominator for free) -> partial output projection wo[:, its 128 cols] @ O.
Host sums the 4 partials per batch, adds bo and the residual.
"""

import sys

if "/opt/trn_rl_repo" not in sys.path:
    sys.path.insert(0, "/opt/trn_rl_repo")

import numpy as np

B, C, H, W = 2, 512, 64, 64
N = H * W              # 4096 tokens
NH, HD = 8, 64         # heads, head_dim
NG = 32                # groupnorm groups (16 channels each)
EPS = 1e-5
N_CORES = 8
P = 128                # partitions
IT = 512               # i-tile (query) width
N_IT = N // IT         # 8
N_JB = N // P          # 32 j-blocks
GROUPS = [3] * 10 + [2]  # jb group sizes per exp batch (sum = 32)


def build_module(do_compile=True, mode=None):
    import os
    import concourse.bass as bass
    import concourse.mybir as mybir
    import concourse.tile as tile
    from concourse import bacc
    from concourse.masks import make_identity

    if mode is None:
        mode = os.environ.get("XATTN_MODE", "bf16")
    assert mode in ("fp32", "bf16")

    f32 = mybir.dt.float32
    # compute dtype for matmul operands: bf16 runs the PE at 1 cycle/row
    # (fp32 matmuls cost 4 cycles/row); accumulation stays fp32 in PSUM.
    dt_c = mybir.dt.bfloat16 if mode == "bf16" else f32
    AF = mybir.ActivationFunctionType
    ALU = mybir.AluOpType

    def mm(out, lhsT, rhs, **kw):
        nc.tensor.matmul(out, lhsT, rhs, **kw)

    nc = bacc.Bacc(name=f"xattn_{mode}")

    x_d = nc.dram_tensor("x", (C, N), f32, kind="ExternalInput")
    wqT_d = nc.dram_tensor("wqT", (C, P), dt_c, kind="ExternalInput")
    wkvT_d = nc.dram_tensor("wkvT", (C, 2 * P), dt_c, kind="ExternalInput")
    woTA_d = nc.dram_tensor("woTA", (HD, C), dt_c, kind="ExternalInput")
    woTB_d = nc.dram_tensor("woTB", (HD, C), dt_c, kind="ExternalInput")
    bq_d = nc.dram_tensor("bq", (P,), f32, kind="ExternalInput")
    bk_d = nc.dram_tensor("bk", (P,), f32, kind="ExternalInput")
    bv_d = nc.dram_tensor("bv", (P,), f32, kind="ExternalInput")
    gnw_d = nc.dram_tensor("gnw", (C,), f32, kind="ExternalInput")
    gnb_d = nc.dram_tensor("gnb", (C,), f32, kind="ExternalInput")
    gmask_d = nc.dram_tensor("gmask", (4, P, NG), f32, kind="ExternalInput")
    gmaskT_d = nc.dram_tensor("gmaskT", (4, NG, P), f32, kind="ExternalInput")
    out_d = nc.dram_tensor("out", (C, N), f32, kind="ExternalOutput")

    with tile.TileContext(nc) as tc:
        with (
            tc.tile_pool(name="const", bufs=1) as const,
            tc.tile_pool(name="qkv", bufs=1) as qkvp,
            tc.tile_pool(name="vt", bufs=1) as vtp,
            tc.tile_pool(name="stx", bufs=1, space="PSUM") as stx,
            tc.tile_pool(name="sty", bufs=1, space="PSUM") as sty,
            tc.tile_pool(name="oa", bufs=1, space="PSUM") as oap,
            tc.tile_pool(name="ob", bufs=1, space="PSUM") as obp,
            tc.tile_pool(name="pta", bufs=3) as pta,
            tc.tile_pool(name="ptb", bufs=3) as ptb,
            tc.tile_pool(name="itn", bufs=2) as itn,
            tc.tile_pool(name="dram", bufs=2, space="DRAM") as dramp,
        ):
            # ---------- constants ----------
            wqT_sb = const.tile([P, 4, P], dt_c, tag="wq")
            nc.sync.dma_start(wqT_sb[:], wqT_d[:].rearrange("(t p) m -> p t m", p=P))
            wkvT_sb = const.tile([P, 4, 2 * P], dt_c, tag="wkv")
            nc.sync.dma_start(wkvT_sb[:], wkvT_d[:].rearrange("(t p) m -> p t m", p=P))
            woTA_sb = const.tile([HD, C], dt_c, tag="woa")
            nc.sync.dma_start(woTA_sb[:], woTA_d[:])
            woTB_sb = const.tile([HD, C], dt_c, tag="wob")
            nc.sync.dma_start(woTB_sb[:], woTB_d[:])
            bq_sb = const.tile([P, 1], f32, tag="bq")
            nc.sync.dma_start(bq_sb[:], bq_d[:, None])
            bk_sb = const.tile([P, 1], f32, tag="bk")
            nc.sync.dma_start(bk_sb[:], bk_d[:, None])
            bv_sb = const.tile([P, 1], f32, tag="bv")
            nc.sync.dma_start(bv_sb[:], bv_d[:, None])
            gnw_sb = const.tile([P, 4], f32, tag="gnw")
            nc.sync.dma_start(gnw_sb[:], gnw_d[:].rearrange("(t p) -> p t", p=P))
            gnb_sb = const.tile([P, 4], f32, tag="gnb")
            nc.sync.dma_start(gnb_sb[:], gnb_d[:].rearrange("(t p) -> p t", p=P))
            gmask_sb = const.tile([P, 4, NG], f32, tag="gmask")
            nc.sync.dma_start(gmask_sb[:], gmask_d[:].rearrange("t p g -> p t g"))
            gmaskT_sb = const.tile([NG, 4, P], f32, tag="gmaskT")
            nc.sync.dma_start(gmaskT_sb[:], gmaskT_d[:].rearrange("t k m -> k t m"))
            ident_sb = const.tile([P, P], dt_c, tag="ident")
            make_identity(nc, ident_sb[:])
            eps_sb = const.tile([NG, 1], f32, tag="eps")
            nc.vector.memset(eps_sb[:], EPS)
            warm = const.tile([1, 2], f32, tag="warm")
            nc.vector.memset(warm[:], 1.0)
            nc.scalar.activation(warm[:, 0:1], warm[:, 0:1], AF.Exp)
            nc.scalar.activation(warm[:, 1:2], warm[:, 1:2], AF.Ln)

            q_sb = qkvp.tile([P, N], dt_c, tag="q")
            k_sb = qkvp.tile([P, N], dt_c, tag="k")
            v_sb = qkvp.tile([P, N], dt_c, tag="v")

            # ---------- phase 1: GroupNorm + QKV projections ----------
            with tc.tile_pool(name="xp", bufs=1) as xp, \
                 tc.tile_pool(name="gn", bufs=1) as gnp:
                x_t = [xp.tile([P, N], f32, tag=f"x{t}", name=f"x{t}")
                       for t in range(4)]
                if mode == "bf16":
                    xs_t = [xp.tile([P, N], dt_c, tag=f"xs{t}", name=f"xs{t}")
                            for t in range(4)]
                else:
                    xs_t = x_t  # normalize in place, fp32 matmul operands
                for t in range(4):
                    nc.sync.dma_start(x_t[t][:], x_d[t * P:(t + 1) * P, :])

                # per-channel stats via bn_stats/bn_aggr (8 subgroups of 512)
                FM = 512
                nsub = N // FM
                BSD = nc.vector.BN_STATS_DIM
                grp_ps = oap.tile([NG, 2], f32, tag="oa")
                for t in range(4):
                    stats = gnp.tile([P, nsub, BSD], f32, tag="bnst")
                    xr = x_t[t][:].rearrange("p (s f) -> p s f", f=FM)
                    for s in range(nsub):
                        nc.vector.bn_stats(stats[:, s, :], xr[:, s, :])
                    mv = gnp.tile([P, 2], f32, tag="mv")
                    nc.vector.bn_aggr(mv[:], stats[:])
                    # stats2: col0 = mean, col1 = E[x^2] = var + mean^2
                    st2 = gnp.tile([P, 2], f32, tag="st2")
                    nc.vector.tensor_mul(st2[:, 1:2], mv[:, 0:1], mv[:, 0:1])
                    nc.vector.tensor_add(st2[:, 1:2], st2[:, 1:2], mv[:, 1:2])
                    nc.vector.tensor_copy(st2[:, 0:1], mv[:, 0:1])
                    # group aggregate: [NG, 2] += gmask_t.T @ st2  (mask = 1/16)
                    nc.tensor.matmul(grp_ps[:], gmask_sb[:, t, :], st2[:],
                                     start=(t == 0), stop=(t == 3))

                # group mu / rstd
                gmv = gnp.tile([NG, 2], f32, tag="gmv")
                nc.vector.tensor_copy(gmv[:], grp_ps[:])
                varg = gnp.tile([NG, 1], f32, tag="varg")
                nc.vector.tensor_mul(varg[:], gmv[:, 0:1], gmv[:, 0:1])
                nc.vector.tensor_tensor(varg[:], gmv[:, 1:2], varg[:],
                                        op=ALU.subtract)
                # rstd = exp(-0.5 * ln(var + eps))
                gvals = gnp.tile([NG, 2], f32, tag="gvals")
                nc.scalar.activation(varg[:], varg[:], AF.Ln, bias=eps_sb[:])
                nc.scalar.activation(gvals[:, 1:2], varg[:], AF.Exp, scale=-0.5)
                nc.vector.tensor_copy(gvals[:, 0:1], gmv[:, 0:1])

                # scatter back to channels: chan = gmaskT_t.T @ gvals -> [128, 2]
                for t in range(4):
                    chan_ps = obp.tile([P, 2], f32, tag="ob")
                    nc.tensor.matmul(chan_ps[:], gmaskT_sb[:, t, :], gvals[:],
                                     start=True, stop=True)
                    s_t = gnp.tile([P, 1], f32, tag=f"s{t}")
                    t_t = gnp.tile([P, 1], f32, tag=f"t{t}")
                    nc.vector.tensor_mul(s_t[:], chan_ps[:, 1:2], gnw_sb[:, t:t + 1])
                    nc.vector.tensor_mul(t_t[:], chan_ps[:, 0:1], s_t[:])
                    nc.vector.tensor_tensor(t_t[:], gnb_sb[:, t:t + 1], t_t[:],
                                            op=ALU.subtract)
                    # xs = x * s + t   (the full normalized+affine input),
                    # cast to the matmul compute dtype on write
                    nc.vector.tensor_scalar(xs_t[t][:], x_t[t][:], s_t[:], t_t[:],
                                            op0=ALU.mult, op1=ALU.add)

                # QKV projections: q = wqT.T @ xs + bq ; [k|v] = wkvT.T @ xs + b
                for n in range(N_IT):
                    nsl = slice(n * IT, (n + 1) * IT)
                    q_ps = (stx if n % 2 == 0 else sty).tile(
                        [P, 3, IT], f32, tag="stx" if n % 2 == 0 else "sty")
                    for t in range(4):
                        mm(q_ps[:, 0, :], wqT_sb[:, t, :],
                           xs_t[t][:, nsl], start=(t == 0), stop=(t == 3))
                    nc.vector.tensor_scalar_add(q_sb[:, nsl], q_ps[:, 0, :],
                                                bq_sb[:])
                for n in range(N_IT):
                    nsl = slice(n * IT, (n + 1) * IT)
                    kv_ps = (stx if n % 2 == 0 else sty).tile(
                        [P, 3, IT], f32, tag="stx" if n % 2 == 0 else "sty")
                    for t in range(4):
                        mm(kv_ps[:, 0, :], wkvT_sb[:, t, 0:P],
                           xs_t[t][:, nsl], start=(t == 0), stop=(t == 3))
                        mm(kv_ps[:, 1, :], wkvT_sb[:, t, P:2 * P],
                           xs_t[t][:, nsl], start=(t == 0), stop=(t == 3))
                    nc.vector.tensor_scalar_add(k_sb[:, nsl], kv_ps[:, 0, :],
                                                bk_sb[:])
                    nc.vector.tensor_scalar_add(v_sb[:, nsl], kv_ps[:, 1, :],
                                                bv_sb[:])

            # ---------- v^T (with trailing ones column for the denominator) ----
            vTA_sb = vtp.tile([P, N_JB, HD + 1], dt_c, tag="vta")
            vTB_sb = vtp.tile([P, N_JB, HD + 1], dt_c, tag="vtb")
            nc.vector.memset(vTA_sb[:, :, HD:HD + 1], 1.0)
            nc.vector.memset(vTB_sb[:, :, HD:HD + 1], 1.0)
            for jb in range(N_JB):
                tp_ps = (oap if jb % 2 == 0 else obp).tile(
                    [P, P], dt_c, tag="oa" if jb % 2 == 0 else "ob")
                nc.tensor.transpose(tp_ps[:], v_sb[:, jb * P:(jb + 1) * P],
                                    ident_sb[:])
                nc.vector.tensor_copy(vTA_sb[:, jb, 0:HD], tp_ps[:, 0:HD])
                nc.vector.tensor_copy(vTB_sb[:, jb, 0:HD], tp_ps[:, HD:P])

            # ---------- phase 2: attention ----------
            o_tiles = {}

            def _emit_o(grp):
                pa, pb, g0, g = grp
                oa_t, ob_t = o_tiles["a"], o_tiles["b"]
                for jj in range(g):
                    jb = g0 + jj
                    mm(oa_t[0:HD + 1, :], vTA_sb[:, jb, :],
                       pa[:, jj, :], start=(jb == 0), stop=(jb == N_JB - 1))
                    mm(ob_t[0:HD + 1, :], vTB_sb[:, jb, :],
                       pb[:, jj, :], start=(jb == 0), stop=(jb == N_JB - 1))

            for it in range(N_IT):
                isl = slice(it * IT, (it + 1) * IT)
                oa_t = oap.tile([P, IT], f32, tag="oa")
                ob_t = obp.tile([P, IT], f32, tag="ob")
                o_tiles["a"], o_tiles["b"] = oa_t, ob_t
                jb0 = 0
                prev = None  # (pa, pb, jb0, gs) of the previous group
                for gs in GROUPS:
                    st_a = stx.tile([P, 3, IT], f32, tag="stx")
                    st_b = sty.tile([P, 3, IT], f32, tag="sty")
                    for jj in range(gs):
                        jb = jb0 + jj
                        jsl = slice(jb * P, (jb + 1) * P)
                        mm(st_a[:, jj, :], k_sb[0:HD, jsl],
                           q_sb[0:HD, isl], start=True, stop=True)
                        mm(st_b[:, jj, :], k_sb[HD:P, jsl],
                           q_sb[HD:P, isl], start=True, stop=True,
                           tile_position=(64, 0))
                    pa = pta.tile([P, 3, IT], dt_c, tag="pta")
                    pb = ptb.tile([P, 3, IT], dt_c, tag="ptb")
                    nc.scalar.activation(pa[:, 0:gs, :], st_a[:, 0:gs, :],
                                         AF.Exp, scale=0.125)
                    nc.scalar.activation(pb[:, 0:gs, :], st_b[:, 0:gs, :],
                                         AF.Exp, scale=0.125)
                    # O matmuls run one group behind the exps, so the PE
                    # never sits in-order-blocked on the current exp and the
                    # next group's score fills keep ScalarE fed.
                    if prev is not None:
                        _emit_o(prev)
                    prev = (pa, pb, jb0, gs)
                    jb0 += gs
                _emit_o(prev)

                # normalization: denominators sit in row HD of oa_t / ob_t
                denA = itn.tile([HD + 1, IT], f32, tag="denA")
                denB = itn.tile([HD + 1, IT], f32, tag="denB")
                ostA = itn.tile([HD, IT], dt_c, tag="ostA")
                ostB = itn.tile([HD, IT], dt_c, tag="ostB")
                nc.vector.tensor_copy(denA[HD:HD + 1, :], oa_t[HD:HD + 1, :])
                nc.vector.tensor_copy(denB[HD:HD + 1, :], ob_t[HD:HD + 1, :])
                nc.vector.reciprocal(denA[HD:HD + 1, :], denA[HD:HD + 1, :])
                nc.vector.reciprocal(denB[HD:HD + 1, :], denB[HD:HD + 1, :])
                scr = dramp.tile([2, IT], f32, tag="scr")
                nc.sync.dma_start(scr[0:1, :], denA[HD:HD + 1, :])
                nc.sync.dma_start(scr[1:2, :], denB[HD:HD + 1, :])
                import concourse.bass as _b
                bcA = itn.tile([HD, IT], f32, tag="bcA")
                bcB = itn.tile([HD, IT], f32, tag="bcB")
                srcA = _b.AP(tensor=scr.tensor, offset=scr.offset,
                             ap=[[0, HD], [1, IT]])
                srcB = _b.AP(tensor=scr.tensor, offset=scr.offset + IT,
                             ap=[[0, HD], [1, IT]])
                nc.sync.dma_start(bcA[:], srcA)
                nc.sync.dma_start(bcB[:], srcB)
                nc.vector.tensor_mul(ostA[:], oa_t[0:HD, :], bcA[:])
                nc.vector.tensor_mul(ostB[:], ob_t[0:HD, :], bcB[:])

                # output projection (K=64 per head, accumulate both heads)
                for mt in range(4):
                    msl = slice(mt * P, (mt + 1) * P)
                    pr_ps = (oap if mt % 2 == 0 else obp).tile(
                        [P, IT], f32, tag="oa" if mt % 2 == 0 else "ob")
                    mm(pr_ps[:], woTA_sb[:, msl], ostA[:],
                       start=True, stop=False)
                    mm(pr_ps[:], woTB_sb[:, msl], ostB[:],
                       start=False, stop=True)
                    pr_sb = itn.tile([P, IT], f32, tag="prsb")
                    nc.vector.tensor_copy(pr_sb[:], pr_ps[:])
                    nc.sync.dma_start(out_d[msl, isl], pr_sb[:])

    if do_compile:
        nc.compile()
    return nc


_CACHE = {}


def _get_runner():
    """Compile once and cache a jitted 8-core SPMD executable."""
    if "runner" in _CACHE:
        return _CACHE["runner"]
    import jax
    import concourse.mybir as mybir
    from concourse.bass2jax import (_bass_exec_p, install_neuronx_cc_hook,
                                    partition_id_tensor)
    from jax.sharding import Mesh, PartitionSpec
    from jax.experimental.shard_map import shard_map

    nc = build_module()
    install_neuronx_cc_hook()
    partition_name = (nc.partition_id_tensor.name
                      if nc.partition_id_tensor else None)
    in_names, out_names, out_avals, zero_outs = [], [], [], []
    for alloc in nc.m.functions[0].allocations:
        if not isinstance(alloc, mybir.MemoryLocationSet):
            continue
        name = alloc.memorylocations[0].name
        if alloc.kind == "ExternalInput":
            if name != partition_name:
                in_names.append(name)
        elif alloc.kind == "ExternalOutput":
            out_names.append(name)
            shape = tuple(alloc.tensor_shape)
            dtype = mybir.dt.np(alloc.dtype)
            out_avals.append(jax.core.ShapedArray(shape, dtype))
            zero_outs.append(np.zeros(shape, dtype))
    n_params = len(in_names)
    n_outs = len(out_avals)
    param_names = list(in_names)
    all_in_names = in_names + out_names
    if partition_name is not None:
        all_in_names.append(partition_name)
    donate = tuple(range(n_params, n_params + n_outs))

    def _body(*args):
        operands = list(args)
        if partition_name is not None:
            operands.append(partition_id_tensor())
        return tuple(_bass_exec_p.bind(
            *operands, out_avals=tuple(out_avals),
            in_names=tuple(all_in_names), out_names=tuple(out_names),
            lowering_input_output_aliases=(),
            sim_require_finite=True, sim_require_nnan=True, nc=nc))

    devices = jax.devices()[:N_CORES]
    mesh = Mesh(np.asarray(devices), ("core",))
    specs = (PartitionSpec("core"),)
    sharded = jax.jit(
        shard_map(_body, mesh=mesh, in_specs=specs * (n_params + n_outs),
                  out_specs=specs * len(out_names), check_rep=False),
        donate_argnums=donate, keep_unused=True)
    oi = out_names.index("out")

    def run(in_maps):
        concat_in = [
            np.concatenate([np.asarray(in_maps[c][name])
                            for c in range(N_CORES)], axis=0)
            for name in param_names
        ]
        concat_zeros = [
            np.zeros((N_CORES * z.shape[0], *z.shape[1:]), z.dtype)
            for z in zero_outs
        ]
        out_arrs = sharded(*concat_in, *concat_zeros)
        full = np.asarray(out_arrs[oi]).reshape(N_CORES, *out_avals[oi].shape)
        return [full[c] for c in range(N_CORES)]

    _CACHE["runner"] = run
    return run


def _masks():
    gmask = np.zeros((4, P, NG), np.float32)
    gmaskT = np.zeros((4, NG, P), np.float32)
    for t in range(4):
        for p in range(P):
            g = (t * P + p) // 16
            gmask[t, p, g] = 1.0 / 16.0
            gmaskT[t, g, p] = 1.0
    return gmask, gmaskT


def kernel(x, gn_w, gn_b, wq, bq, wkv, bkv, wo, bo):
    x = np.asarray(x, np.float32)
    gn_w = np.asarray(gn_w, np.float32)
    gn_b = np.asarray(gn_b, np.float32)
    wq = np.asarray(wq, np.float32)
    bq = np.asarray(bq, np.float32)
    wkv = np.asarray(wkv, np.float32)
    bkv = np.asarray(bkv, np.float32)
    wo = np.asarray(wo, np.float32)
    bo = np.asarray(bo, np.float32)

    import os
    import ml_dtypes
    wdt = (np.dtype(ml_dtypes.bfloat16)
           if os.environ.get("XATTN_MODE", "bf16") == "bf16" else np.float32)
    gmask, gmaskT = _masks()
    xf = x.reshape(B, C, N)
    in_maps = []
    for core in range(N_CORES):
        b = core // 4
        ho = (core % 4) * 2
        rows = slice(ho * HD, ho * HD + P)
        wkv_h = np.concatenate([wkv[ho * HD:ho * HD + P, :],
                                wkv[C + ho * HD:C + ho * HD + P, :]], axis=0)
        wo_h = wo[:, rows]  # (C, 128)
        in_maps.append({
            "x": np.ascontiguousarray(xf[b]),
            "wqT": np.ascontiguousarray(wq[rows, :].T).astype(wdt),
            "wkvT": np.ascontiguousarray(wkv_h.T).astype(wdt),
            "woTA": np.ascontiguousarray(wo_h[:, 0:HD].T).astype(wdt),
            "woTB": np.ascontiguousarray(wo_h[:, HD:P].T).astype(wdt),
            "bq": np.ascontiguousarray(bq[rows]),
            "bk": np.ascontiguousarray(bkv[ho * HD:ho * HD + P]),
            "bv": np.ascontiguousarray(bkv[C + ho * HD:C + ho * HD + P]),
            "gnw": gn_w, "gnb": gn_b,
            "gmask": gmask, "gmaskT": gmaskT,
        })

    partials = _get_runner()(in_maps)
    out = np.empty((B, C, N), np.float32)
    for b in range(B):
        acc = partials[4 * b].astype(np.float32)
        for c in range(1, 4):
            acc = acc + partials[4 * b + c]
        out[b] = acc + bo[:, None] + xf[b]
    return out.reshape(B, C, H, W)
